# revision 32
# baseline (speedup 1.0000x reference)
"""BioBERT entity-aware enhancement kernel for 8 TRN2 NeuronCores.

Math reformulation (ent_tokens is batch-independent):
    M[s, e] = #{t : ent_tokens[e, t] == s}            (fixed [S, E] count matrix)
    entity_emb[b] = A[b]^T @ W                         A[b] = [onehot(types)|conf|1]  [7, E]
                                                       W    = [type_table; conf_w; conf_b]  [7, H]
    enhanced[b]   = hidden[b] + M @ entity_emb[b]
    entity_out[b] = 0.25 * (M^T @ hidden[b] + (M^T M) @ entity_emb[b])

The pooled output uses the M^T M decomposition so it depends only on the
hidden load + the (tiny, hidden-independent) entity embeddings — the
enhance and pool branches run concurrently per batch.

Data-parallel over batch: each of the 8 cores handles B/8 = 8 batches.
Per-core traffic: 12.6 MiB in + 15.7 MiB out -> memory-bound.

TensorEngine operands are bf16 (fp32 matmul runs at ~1/4 rate via hi/lo
passes); accumulation is f32 in PSUM and the dominant f32 hidden term is
added on the VectorEngine in f32, so `enhanced` stays near-exact.

Sequence rows are laid out partition-major (s = p*4 + n) so every DMA
moves 12 KiB contiguous per partition; M is host-permuted to match.
"""

import ml_dtypes
import numpy as np

import concourse.bacc as bacc
import concourse.mybir as mybir
import concourse.tile as tile
from concourse.bass_utils import run_bass_kernel_spmd

B, S, H = 64, 512, 768
E, T = 128, 4
N_CORES = 8
PB = B // N_CORES          # batches per core
P = 128                    # SBUF partitions
NS = S // P                # 4 sequence-groups: s = p*NS + n
HC = 384                   # H split into 2 chunks (psum bank = 512 f32)
NH = H // HC
F32 = mybir.dt.float32
BF16 = mybir.dt.bfloat16
NP_BF16 = ml_dtypes.bfloat16

_cache = {}


def _build_nc():
    nc = bacc.Bacc("TRN2", target_bir_lowering=False, debug=False)

    hid_d = nc.dram_tensor("hidden", [PB, S, H], F32, kind="ExternalInput").ap()
    at_d = nc.dram_tensor("a_t", [PB, 7, E], BF16, kind="ExternalInput").ap()
    w_d = nc.dram_tensor("w", [7, H], BF16, kind="ExternalInput").ap()
    mt_d = nc.dram_tensor("mtg", [E, NS, P], BF16, kind="ExternalInput").ap()
    mq_d = nc.dram_tensor("mqg", [P, NS, E], BF16, kind="ExternalInput").ap()
    mn_d = nc.dram_tensor("mng", [P, NS, E], BF16, kind="ExternalInput").ap()
    enh_d = nc.dram_tensor("enhanced", [PB, S, H], F32, kind="ExternalOutput").ap()
    ent_d = nc.dram_tensor("ent_out", [PB, E, H], F32, kind="ExternalOutput").ap()

    with tile.TileContext(nc) as tc:
        with (
            tc.tile_pool(name="const", bufs=1) as cpool,
            tc.tile_pool(name="hid", bufs=6) as hidp,
            tc.tile_pool(name="out", bufs=3) as outp,
            tc.tile_pool(name="hidbf", bufs=3) as hidbfp,
            tc.tile_pool(name="ent", bufs=2) as entp,
            tc.tile_pool(name="pooled", bufs=3) as poolp,
            tc.tile_pool(name="ps_ent", bufs=1, space="PSUM") as ps_ent,
            tc.tile_pool(name="ps_enh", bufs=5, space="PSUM") as ps_enh,
            tc.tile_pool(name="ps_pool", bufs=2, space="PSUM") as ps_pool,
        ):
            # constants, loaded once (SWDGE ring so the big hidden loads
            # on the HWDGE rings start immediately)
            mt_sb = cpool.tile([E, NS, P], BF16)     # M^T groups   [e, n, p]
            nc.gpsimd.dma_start(out=mt_sb[:], in_=mt_d[:])
            mq_sb = cpool.tile([P, NS, E], BF16)     # 0.25*M       [p, n, e]
            nc.gpsimd.dma_start(out=mq_sb[:], in_=mq_d[:])
            mn_sb = cpool.tile([P, NS, E], BF16)     # M (unscaled) [p, n, e]
            nc.gpsimd.dma_start(out=mn_sb[:], in_=mn_d[:])
            w_sb = cpool.tile([7, H], BF16)
            nc.gpsimd.dma_start(out=w_sb[:], in_=w_d[:])
            at_sb = cpool.tile([7, PB, E], BF16)     # all batches at once
            nc.gpsimd.dma_start(out=at_sb[:], in_=at_d.rearrange("b k e -> k b e"))

            # 0.25 * M^T M  [E, E], device-precomputed once
            mtm_sb = cpool.tile([E, E], BF16)
            ps = ps_ent.tile([E, E], F32, tag="pse")
            for n in range(NS):
                nc.tensor.matmul(
                    ps[:], mn_sb[:, n, :], mq_sb[:, n, :],
                    start=(n == 0), stop=(n == NS - 1),
                )
            nc.scalar.copy(mtm_sb[:], ps[:])

            for b in range(PB):
                hid_sb = hidp.tile([P, NS, H], F32, tag="hid")
                nc.sync.dma_start(
                    out=hid_sb[:],
                    in_=hid_d[b].rearrange("(p n) h -> p n h", n=NS),
                )

                # entity_emb = A^T W    [E, H]
                ent_sb = entp.tile([E, H], BF16, tag="ent")
                for c in range(NH):
                    ps = ps_ent.tile([E, HC], F32, tag="pse")
                    nc.tensor.matmul(
                        ps[:], at_sb[:, b, :], w_sb[:, c * HC:(c + 1) * HC],
                        start=True, stop=True,
                    )
                    nc.scalar.copy(ent_sb[:, c * HC:(c + 1) * HC], ps[:])

                # enhanced = hidden + M @ entity_emb
                out_sb = outp.tile([P, NS, H], F32, tag="out")
                for n in range(NS):
                    for c in range(NH):
                        ps = ps_enh.tile([P, HC], F32, tag="psh")
                        nc.tensor.matmul(
                            ps[:],
                            mt_sb[:, n, :],
                            ent_sb[:, c * HC:(c + 1) * HC],
                            start=True, stop=True,
                        )
                        nc.vector.tensor_add(
                            out=out_sb[:, n, c * HC:(c + 1) * HC],
                            in0=hid_sb[:, n, c * HC:(c + 1) * HC],
                            in1=ps[:],
                        )
                enh_view = enh_d[b].rearrange("(p n) h -> p n h", n=NS)
                if b >= PB - 2:
                    # tail batches: store per half so the final store
                    # stream starts draining earlier
                    nc.gpsimd.dma_start(out=enh_view[:, :2], in_=out_sb[:, :2])
                    nc.gpsimd.dma_start(out=enh_view[:, 2:], in_=out_sb[:, 2:])
                else:
                    nc.gpsimd.dma_start(out=enh_view, in_=out_sb[:])

                # bf16 shadow of hidden for the pooling matmul — after the
                # adds so the store-critical DVE work is not delayed
                # (split between ACT and DVE to balance engine load)
                hid_bf = hidbfp.tile([P, NS, H], BF16, tag="hidbf")
                if b % 2 == 0:
                    nc.scalar.copy(hid_bf[:], hid_sb[:])
                else:
                    nc.vector.tensor_copy(out=hid_bf[:], in_=hid_sb[:])

                # entity_out = 0.25 * (M^T hidden + M^T M entity_emb)
                pool_sb = poolp.tile([E, H], F32, tag="pooled")
                for c in range(NH):
                    ps = ps_pool.tile([E, HC], F32, tag="psp")
                    for n in range(NS):
                        nc.tensor.matmul(
                            ps[:],
                            mq_sb[:, n, :],
                            hid_bf[:, n, c * HC:(c + 1) * HC],
                            start=(n == 0), stop=False,
                        )
                    nc.tensor.matmul(
                        ps[:],
                        mtm_sb[:],
                        ent_sb[:, c * HC:(c + 1) * HC],
                        start=False, stop=True,
                    )
                    nc.scalar.copy(pool_sb[:, c * HC:(c + 1) * HC], ps[:])
                nc.scalar.dma_start(out=ent_d[b], in_=pool_sb[:])

    nc.compile()
    return nc


def _get_nc():
    if "nc" not in _cache:
        _cache["nc"] = _build_nc()
    return _cache["nc"]


def _prepare_in_maps(hidden_states, entity_types, entity_confidences, ent_tokens,
                     type_table, conf_w, conf_b):
    hidden = np.ascontiguousarray(np.asarray(hidden_states, np.float32))
    types = np.asarray(entity_types, np.int32)
    conf = np.asarray(entity_confidences, np.float32)
    toks = np.asarray(ent_tokens, np.int32)
    ttab = np.asarray(type_table, np.float32)
    cw = np.asarray(conf_w, np.float32)
    cb = np.asarray(conf_b, np.float32)

    m = np.zeros((S, E), np.float32)
    np.add.at(m, (toks.reshape(-1), np.repeat(np.arange(E), T)), 1.0)
    m3 = m.reshape(P, NS, E)                      # [p, n, e], s = p*NS + n
    mtg = np.ascontiguousarray(m3.transpose(2, 1, 0)).astype(NP_BF16)   # [E, NS, P]
    mqg = np.ascontiguousarray(0.25 * m3).astype(NP_BF16)               # [P, NS, E]
    mng = np.ascontiguousarray(m3).astype(NP_BF16)                      # [P, NS, E]

    w = np.concatenate([ttab, cw.reshape(1, H), cb[None]], 0).astype(NP_BF16)
    a_t = np.zeros((B, 7, E), np.float32)
    a_t[np.arange(B)[:, None], types, np.arange(E)[None, :]] = 1.0
    a_t[:, 5, :] = conf
    a_t[:, 6, :] = 1.0
    a_t = a_t.astype(NP_BF16)

    hid_sh = hidden.reshape(N_CORES, PB, S, H)
    at_sh = np.ascontiguousarray(a_t.reshape(N_CORES, PB, 7, E))
    return [
        {"hidden": hid_sh[i], "a_t": at_sh[i], "w": w,
         "mtg": mtg, "mqg": mqg, "mng": mng}
        for i in range(N_CORES)
    ]


def _run(in_maps, **kwargs):
    nc = _get_nc()
    return run_bass_kernel_spmd(nc, in_maps, core_ids=list(range(N_CORES)), **kwargs)


def _assemble(results):
    enhanced = np.concatenate(
        [results[i]["enhanced"] for i in range(N_CORES)], 0
    ).reshape(B, S, H)
    ent_out = np.concatenate(
        [results[i]["ent_out"] for i in range(N_CORES)], 0
    ).reshape(B, E, H)
    return enhanced, ent_out


def kernel(**inputs):
    in_maps = _prepare_in_maps(**inputs)
    res = _run(in_maps)
    return _assemble(res.results)


def kernel_profiled(**inputs):
    """Same as kernel() but with NTFF tracing; returns (outputs, BassKernelResults)."""
    in_maps = _prepare_in_maps(**inputs)
    res = _run(in_maps, trace=True)
    return _assemble(res.results), res


# revision 34
# speedup vs baseline: 1.0314x; 1.0314x over previous
"""BioBERT entity-aware enhancement kernel for 8 TRN2 NeuronCores.

Math reformulation (ent_tokens is batch-independent):
    M[s, e] = #{t : ent_tokens[e, t] == s}            (fixed [S, E] count matrix)
    entity_emb[b] = A[b]^T @ W                         A[b] = [onehot(types)|conf|1]  [7, E]
                                                       W    = [type_table; conf_w; conf_b]  [7, H]
    enhanced[b]   = hidden[b] + M @ entity_emb[b]
    entity_out[b] = 0.25 * (M^T @ hidden[b] + (M^T M) @ entity_emb[b])

The pooled output uses the M^T M decomposition so it depends only on the
hidden load + the (tiny, hidden-independent) entity embeddings — the
enhance and pool branches run concurrently per batch.

Data-parallel over batch: each of the 8 cores handles B/8 = 8 batches.
Per-core traffic: 12.6 MiB in + 15.7 MiB out -> memory-bound.

TensorEngine operands are bf16 (fp32 matmul runs at ~1/4 rate via hi/lo
passes); accumulation is f32 in PSUM and the dominant f32 hidden term is
added on the VectorEngine in f32, so `enhanced` stays near-exact.

Sequence rows are laid out partition-major (s = p*4 + n) so every DMA
moves 12 KiB contiguous per partition; M is host-permuted to match.
"""

import ml_dtypes
import numpy as np

import concourse.bacc as bacc
import concourse.mybir as mybir
import concourse.tile as tile
from concourse.bass_utils import run_bass_kernel_spmd

B, S, H = 64, 512, 768
E, T = 128, 4
N_CORES = 8
PB = B // N_CORES          # batches per core
P = 128                    # SBUF partitions
NS = S // P                # 4 sequence-groups: s = p*NS + n
HC = 384                   # H split into 2 chunks (psum bank = 512 f32)
NH = H // HC
F32 = mybir.dt.float32
BF16 = mybir.dt.bfloat16
NP_BF16 = ml_dtypes.bfloat16

_cache = {}


def _build_nc():
    nc = bacc.Bacc("TRN2", target_bir_lowering=False, debug=False)

    hid_d = nc.dram_tensor("hidden", [PB, S, H], F32, kind="ExternalInput").ap()
    at_d = nc.dram_tensor("a_t", [PB, 7, E], BF16, kind="ExternalInput").ap()
    w_d = nc.dram_tensor("w", [7, H], BF16, kind="ExternalInput").ap()
    mt_d = nc.dram_tensor("mtg", [E, NS, P], BF16, kind="ExternalInput").ap()
    mq_d = nc.dram_tensor("mqg", [P, NS, E], BF16, kind="ExternalInput").ap()
    mn_d = nc.dram_tensor("mng", [P, NS, E], BF16, kind="ExternalInput").ap()
    enh_d = nc.dram_tensor("enhanced", [PB, S, H], F32, kind="ExternalOutput").ap()
    ent_d = nc.dram_tensor("ent_out", [PB, E, H], F32, kind="ExternalOutput").ap()

    with tile.TileContext(nc) as tc:
        with (
            tc.tile_pool(name="const", bufs=1) as cpool,
            tc.tile_pool(name="hid", bufs=6) as hidp,
            tc.tile_pool(name="out", bufs=3) as outp,
            tc.tile_pool(name="hidbf", bufs=3) as hidbfp,
            tc.tile_pool(name="ent", bufs=2) as entp,
            tc.tile_pool(name="pooled", bufs=3) as poolp,
            tc.tile_pool(name="ps_ent", bufs=1, space="PSUM") as ps_ent,
            tc.tile_pool(name="ps_enh", bufs=5, space="PSUM") as ps_enh,
            tc.tile_pool(name="ps_pool", bufs=2, space="PSUM") as ps_pool,
        ):
            # constants, loaded once (SWDGE ring so the big hidden loads
            # on the HWDGE rings start immediately)
            mt_sb = cpool.tile([E, NS, P], BF16)     # M^T groups   [e, n, p]
            nc.gpsimd.dma_start(out=mt_sb[:], in_=mt_d[:])
            mq_sb = cpool.tile([P, NS, E], BF16)     # 0.25*M       [p, n, e]
            nc.gpsimd.dma_start(out=mq_sb[:], in_=mq_d[:])
            mn_sb = cpool.tile([P, NS, E], BF16)     # M (unscaled) [p, n, e]
            nc.gpsimd.dma_start(out=mn_sb[:], in_=mn_d[:])
            w_sb = cpool.tile([7, H], BF16)
            nc.gpsimd.dma_start(out=w_sb[:], in_=w_d[:])
            at_sb = cpool.tile([7, PB, E], BF16)     # all batches at once
            nc.gpsimd.dma_start(out=at_sb[:], in_=at_d.rearrange("b k e -> k b e"))

            # 0.25 * M^T M  [E, E], device-precomputed once
            mtm_sb = cpool.tile([E, E], BF16)
            ps = ps_ent.tile([E, E], F32, tag="pse")
            for n in range(NS):
                nc.tensor.matmul(
                    ps[:], mn_sb[:, n, :], mq_sb[:, n, :],
                    start=(n == 0), stop=(n == NS - 1),
                )
            nc.scalar.copy(mtm_sb[:], ps[:])

            for b in range(PB):
                hid_sb = hidp.tile([P, NS, H], F32, tag="hid")
                nc.sync.dma_start(
                    out=hid_sb[:],
                    in_=hid_d[b].rearrange("(p n) h -> p n h", n=NS),
                )

                # bf16 shadow of hidden for the pooling matmul
                # (split between ACT and DVE to balance engine load)
                hid_bf = hidbfp.tile([P, NS, H], BF16, tag="hidbf")
                if b % 2 == 0:
                    nc.scalar.copy(hid_bf[:], hid_sb[:])
                else:
                    nc.vector.tensor_copy(out=hid_bf[:], in_=hid_sb[:])

                # entity_emb = A^T W    [E, H]
                ent_sb = entp.tile([E, H], BF16, tag="ent")
                for c in range(NH):
                    ps = ps_ent.tile([E, HC], F32, tag="pse")
                    nc.tensor.matmul(
                        ps[:], at_sb[:, b, :], w_sb[:, c * HC:(c + 1) * HC],
                        start=True, stop=True,
                    )
                    nc.scalar.copy(ent_sb[:, c * HC:(c + 1) * HC], ps[:])

                # enhanced = hidden + M @ entity_emb
                out_sb = outp.tile([P, NS, H], F32, tag="out")
                for n in range(NS):
                    for c in range(NH):
                        ps = ps_enh.tile([P, HC], F32, tag="psh")
                        nc.tensor.matmul(
                            ps[:],
                            mt_sb[:, n, :],
                            ent_sb[:, c * HC:(c + 1) * HC],
                            start=True, stop=True,
                        )
                        nc.vector.tensor_add(
                            out=out_sb[:, n, c * HC:(c + 1) * HC],
                            in0=hid_sb[:, n, c * HC:(c + 1) * HC],
                            in1=ps[:],
                        )
                enh_view = enh_d[b].rearrange("(p n) h -> p n h", n=NS)
                if b >= PB - 2:
                    # tail batches: store per half so the final store
                    # stream starts draining earlier
                    nc.gpsimd.dma_start(out=enh_view[:, :2], in_=out_sb[:, :2])
                    nc.gpsimd.dma_start(out=enh_view[:, 2:], in_=out_sb[:, 2:])
                else:
                    nc.gpsimd.dma_start(out=enh_view, in_=out_sb[:])

                # entity_out = 0.25 * (M^T hidden + M^T M entity_emb)
                pool_sb = poolp.tile([E, H], F32, tag="pooled")
                for c in range(NH):
                    ps = ps_pool.tile([E, HC], F32, tag="psp")
                    for n in range(NS):
                        nc.tensor.matmul(
                            ps[:],
                            mq_sb[:, n, :],
                            hid_bf[:, n, c * HC:(c + 1) * HC],
                            start=(n == 0), stop=False,
                        )
                    nc.tensor.matmul(
                        ps[:],
                        mtm_sb[:],
                        ent_sb[:, c * HC:(c + 1) * HC],
                        start=False, stop=True,
                    )
                    nc.scalar.copy(pool_sb[:, c * HC:(c + 1) * HC], ps[:])
                nc.scalar.dma_start(out=ent_d[b], in_=pool_sb[:])

    nc.compile()
    return nc


def _get_nc():
    if "nc" not in _cache:
        _cache["nc"] = _build_nc()
    return _cache["nc"]


def _prepare_in_maps(hidden_states, entity_types, entity_confidences, ent_tokens,
                     type_table, conf_w, conf_b):
    hidden = np.ascontiguousarray(np.asarray(hidden_states, np.float32))
    types = np.asarray(entity_types, np.int32)
    conf = np.asarray(entity_confidences, np.float32)
    toks = np.asarray(ent_tokens, np.int32)
    ttab = np.asarray(type_table, np.float32)
    cw = np.asarray(conf_w, np.float32)
    cb = np.asarray(conf_b, np.float32)

    m = np.zeros((S, E), np.float32)
    np.add.at(m, (toks.reshape(-1), np.repeat(np.arange(E), T)), 1.0)
    m3 = m.reshape(P, NS, E)                      # [p, n, e], s = p*NS + n
    mtg = np.ascontiguousarray(m3.transpose(2, 1, 0)).astype(NP_BF16)   # [E, NS, P]
    mqg = np.ascontiguousarray(0.25 * m3).astype(NP_BF16)               # [P, NS, E]
    mng = np.ascontiguousarray(m3).astype(NP_BF16)                      # [P, NS, E]

    w = np.concatenate([ttab, cw.reshape(1, H), cb[None]], 0).astype(NP_BF16)
    a_t = np.zeros((B, 7, E), np.float32)
    a_t[np.arange(B)[:, None], types, np.arange(E)[None, :]] = 1.0
    a_t[:, 5, :] = conf
    a_t[:, 6, :] = 1.0
    a_t = a_t.astype(NP_BF16)

    hid_sh = hidden.reshape(N_CORES, PB, S, H)
    at_sh = np.ascontiguousarray(a_t.reshape(N_CORES, PB, 7, E))
    return [
        {"hidden": hid_sh[i], "a_t": at_sh[i], "w": w,
         "mtg": mtg, "mqg": mqg, "mng": mng}
        for i in range(N_CORES)
    ]


def _run(in_maps, **kwargs):
    nc = _get_nc()
    return run_bass_kernel_spmd(nc, in_maps, core_ids=list(range(N_CORES)), **kwargs)


def _assemble(results):
    enhanced = np.concatenate(
        [results[i]["enhanced"] for i in range(N_CORES)], 0
    ).reshape(B, S, H)
    ent_out = np.concatenate(
        [results[i]["ent_out"] for i in range(N_CORES)], 0
    ).reshape(B, E, H)
    return enhanced, ent_out


def kernel(**inputs):
    in_maps = _prepare_in_maps(**inputs)
    res = _run(in_maps)
    return _assemble(res.results)


def kernel_profiled(**inputs):
    """Same as kernel() but with NTFF tracing; returns (outputs, BassKernelResults)."""
    in_maps = _prepare_in_maps(**inputs)
    res = _run(in_maps, trace=True)
    return _assemble(res.results), res
